# revision 12
# baseline (speedup 1.0000x reference)
"""Trainium2 Bass kernel for an attention-LSTM decoder (Bahdanau attention +
LSTM cell + vocab projection, 20 decode steps), sharded across 8 NeuronCores.

Sharding (core j of 8):
  - out_w vocab-sharded: each core owns 4000 vocab columns (bf16, SBUF-resident).
  - LSTM gates sharded by hidden slice: core j computes gate columns
    [i_j|f_j|g_j|o_j] (4 x 128) -> its h2/c slice; h2^T is AllGathered.
  - attention scores l-sharded (8 of 64 attention pixels per core).
  - attention context f-sharded (256 of 2048 feature dims per core),
    computed via sum_l diag(alpha[:,l]) @ X[:,l,fslice] on the PE.
Activations are kept transposed ([feat, batch]) so every matmul uses the
natural [in,out] weight layout with lhsT = activation^T.
"""

import os
import sys

import numpy as np
import ml_dtypes

sys.path.insert(0, "/opt/trn_rl_repo")

import concourse.bass as bass
import concourse.bacc as bacc
import concourse.mybir as mybir
import concourse.tile as tile
from concourse.bass_utils import run_bass_kernel_spmd

BF16 = ml_dtypes.bfloat16
F32 = mybir.dt.float32
BF = mybir.dt.bfloat16
F32R = mybir.dt.float32r
AF = mybir.ActivationFunctionType
ALU = mybir.AluOpType

NC = 8
B = 128          # batch
L = 64           # attention pixels
FF = 2048        # feature dim
A = 512          # attn dim
E = 512          # embed dim
H = 1024         # hidden
V = 32000        # vocab
T = int(os.environ.get("KERNEL_T", "20"))  # decode steps
LS = L // NC     # l-shard per core (8)
FS = FF // NC    # f-shard per core (256)
HS = H // NC     # hidden slice per core (128)
G = 4 * HS       # gate cols per core (512)
VS = V // NC     # vocab shard per core (4000)
KE = E // 128    # 4
KF = FF // 128   # 16
KH = H // 128    # 8
KEF = (E + FF) // 128  # 20
NLOG = 8         # logits chunks per step
VC = VS // NLOG  # 500 cols per logits chunk

LAST_RESULT = None


def _bc(ap, shape):
    """Broadcast an AP to `shape` by inserting/stretching singleton dims."""
    import concourse.ap_utils as ap_utils
    cur = list(ap.shape)
    # insert singleton dims to match rank
    while len(cur) < len(shape):
        ap = ap.rearrange(
            " ".join(f"d{i}" for i in range(len(cur)))
            + " -> "
            + " ".join(["d0", "1"] + [f"d{i}" for i in range(1, len(cur))])
        )
        cur = list(ap.shape)
    return ap.broadcast_to(shape) if hasattr(ap, "broadcast_to") else ap


def _build(nc):
    dram = {}

    def din(name, shape, dt):
        dram[name] = nc.dram_tensor(name, list(shape), dt, kind="ExternalInput").ap()
        return dram[name]

    def dout(name, shape, dt):
        dram[name] = nc.dram_tensor(name, list(shape), dt, kind="ExternalOutput").ap()
        return dram[name]

    # ---- external inputs (per-core data differs; program identical) ----
    xsT = din("xsT", (T, KE, 128, B), BF)          # emb[y]^T per step, ktiled
    xt_in = din("xt_in", (LS, 128, KF, B), F32R)    # X[:, lj, :]^T  [l, p, kt, b]
    xctx = din("xctx", (B, L, FS), BF)             # X[:, :, fj] for context
    wih = din("wih", (KEF, 128, G), BF)            # w_ih rows x gate-shard cols
    whh = din("whh", (KH, 128, G), BF)
    oww = din("oww", (KH, 128, VS), BF)            # out_w ktiles x vocab shard
    fc1 = din("fc1", (KH, 128, A), BF)
    fc2 = din("fc2", (KF, 128, A), F32R)
    smw = din("smw", (KH, 128, KF, B), F32)        # sm_w [mt, p, kt, b(=h cols)]
    lmw = din("lmw", (KF, 128, HS), F32)           # lm_w cols hj, ktiled
    swr = din("swr", (1, A), F32R)                  # score_w as row
    ide_f = din("ide_f", (128, 128), F32)
    ide_b = din("ide_b", (128, 128), BF)
    ones_f = din("ones_f", (1, 128), F32)
    ones_r = din("ones_r", (1, 128), F32R)
    ones_b = din("ones_b", (1, 128), BF)
    b_fc1 = din("b_fc1", (1, A), F32R)
    b_fc2 = din("b_fc2", (1, A), F32R)
    b_ih = din("b_ih", (1, G), F32R)
    b_hh = din("b_hh", (1, G), F32R)
    b_out = din("b_out", (1, VS), BF)
    b_sm = din("b_sm", (1, H), F32)
    b_lm = din("b_lm", (1, HS), F32)
    b_sc = din("b_sc", (1, 1), F32)

    logits_out = dout("logits_out", (T, B, VS), F32)
    alphas_out = dout("alphas_out", (T, B, L), F32)

    with tile.TileContext(nc) as tc:
        _emit(nc, tc, dram)
    return nc


def _emit(nc, tc, d):
    rg = [list(range(NC))]
    f32r = lambda ap: ap.bitcast(F32R)
    KG = 4          # fc2 ktile group size for the hoist
    NKG = KF // KG  # 4 groups

    with (
        tc.tile_pool(name="consts", bufs=1) as cpool,
        tc.tile_pool(name="weights", bufs=1) as wpool,
        tc.tile_pool(name="state", bufs=1) as spool,
        tc.tile_pool(name="dramb", bufs=2, space="DRAM") as db,
    ):
        # ---------- constants (persistent) ----------
        idef = cpool.tile([128, 128], F32, name="idef")
        ideb = cpool.tile([128, 128], BF, name="ideb")
        onesf = cpool.tile([1, 128], F32, name="onesf")
        onesb = cpool.tile([1, 128], BF, name="onesb")
        onesr = cpool.tile([1, 128], F32R, name="onesr")
        bfc1 = cpool.tile([1, A], F32R, name="bfc1")
        bih = cpool.tile([1, G], F32R, name="bih")
        bhh = cpool.tile([1, G], F32R, name="bhh")
        bout = cpool.tile([1, VS], BF, name="bout")
        swb = cpool.tile([128, A], F32, name="swb")
        sbb = cpool.tile([128, 1], F32, name="sbb")
        for t_, src_ in [
            (idef, d["ide_f"]), (ideb, d["ide_b"]), (onesf, d["ones_f"]),
            (onesb, d["ones_b"]), (onesr, d["ones_r"]), (bfc1, d["b_fc1"]),
            (bih, d["b_ih"]), (bhh, d["b_hh"]), (bout, d["b_out"]),
        ]:
            nc.sync.dma_start(t_[:], src_[:])

        # ---------- resident weights ----------
        OW = wpool.tile([128, KH, VS], BF, name="OW")
        WIH = wpool.tile([128, KEF, G], BF, name="WIH")
        WHH = wpool.tile([128, KH, G], BF, name="WHH")
        FC1 = wpool.tile([128, KH, A], BF, name="FC1")
        XCTX = wpool.tile([128, L, FS], BF, name="XCTX")
        XENC = wpool.tile([128, LS, A], F32, name="XENC")
        for k in range(KH):
            nc.sync.dma_start(OW[:, k, :], d["oww"][k])
            nc.sync.dma_start(WHH[:, k, :], d["whh"][k])
            nc.sync.dma_start(FC1[:, k, :], d["fc1"][k])
        for k in range(KEF):
            nc.sync.dma_start(WIH[:, k, :], d["wih"][k])
        nc.sync.dma_start(XCTX[:], d["xctx"][:])

        # ---------- state ----------
        c_loc = spool.tile([128, HS], F32, name="c_loc")

        def new_h2t(i):
            return spool.tile([128, KH, 128], BF, name=f"h2t_{i}",
                              tag="h2t", bufs=2)

        # ================= hoist =================
        with tc.tile_pool(name="hoistmid", bufs=1) as hm:
            meanT = hm.tile([128, KF, B], F32, name="meanT")
            h2t0 = new_h2t("init")

            with (
                tc.tile_pool(name="hoistA", bufs=1) as hp,
                tc.tile_pool(name="hoistAps", bufs=8, space="PSUM") as hps,
            ):
                bfc2 = hp.tile([1, A], F32R, name="bfc2")
                nc.sync.dma_start(bfc2[:], d["b_fc2"][:])
                # x_enc for the local l-shard + mean partial; fc2 streamed in
                # ktile groups, X^T streamed in (l, group) chunks
                xe_ps = [hps.tile([128, A], F32, name=f"xe_{li}", tag="xe")
                         for li in range(LS)]
                for kg in range(NKG):
                    fc2g = hp.tile([128, KG, A], F32R, name=f"fc2g_{kg}",
                                   tag="fc2g", bufs=2)
                    for k in range(KG):
                        nc.sync.dma_start(fc2g[:, k, :], d["fc2"][kg * KG + k])
                    for li in range(LS):
                        xtc = hp.tile([128, KG, B], F32R,
                                      name=f"xt_{kg}_{li}", tag="xt", bufs=2)
                        nc.sync.dma_start(
                            xtc[:],
                            d["xt_in"][li, :, kg * KG:(kg + 1) * KG, :])
                        for k in range(KG):
                            nc.tensor.matmul(xe_ps[li][:], xtc[:, k, :],
                                             fc2g[:, k, :],
                                             start=(kg == 0 and k == 0),
                                             stop=False)
                        sl = meanT[:, kg * KG:(kg + 1) * KG, :]
                        xtc_f = xtc[:].bitcast(F32)
                        if li == 0:
                            nc.vector.tensor_copy(sl, xtc_f)
                        else:
                            nc.vector.tensor_tensor(sl, sl, xtc_f,
                                                    op=ALU.add)
                for li in range(LS):
                    nc.tensor.matmul(xe_ps[li][:], onesr[:],
                                     bfc2[:], start=False, stop=True)
                    nc.vector.tensor_copy(XENC[:, li, :], xe_ps[li][:])

            # scale by 1/64 and AllReduce across cores
            nc.scalar.mul(meanT[:], meanT[:], 1.0 / L)
            mean_in = db.tile([KF * 128, B], F32, name="mean_in")
            mean_out = db.tile([KF * 128, B], F32, name="mean_out",
                               addr_space="Shared")
            for k in range(KF):
                nc.sync.dma_start(mean_in[k * 128:(k + 1) * 128, :],
                                  meanT[:, k, :])
            nc.gpsimd.collective_compute(
                "AllReduce", ALU.add, replica_groups=rg,
                ins=[mean_in[:]], outs=[mean_out[:]])
            for k in range(KF):
                nc.sync.dma_start(meanT[:, k, :],
                                  mean_out[k * 128:(k + 1) * 128, :])

            with (
                tc.tile_pool(name="hoistB", bufs=1) as hp,
                tc.tile_pool(name="hoistBps", bufs=2, space="PSUM") as hps,
            ):
                bsm = hp.tile([1, H], F32, name="bsm")
                blm = hp.tile([1, HS], F32, name="blm")
                bsc_row = hp.tile([1, 1], F32, name="bsc_row")
                swrow = hp.tile([1, A], F32R, name="swrow")
                for t_, src_ in [(bsm, d["b_sm"]), (blm, d["b_lm"]),
                                 (bsc_row, d["b_sc"]), (swrow, d["swr"])]:
                    nc.sync.dma_start(t_[:], src_[:])

                # score_w / score_b broadcast across partitions
                ps_bc = hps.tile([128, A], F32, name="ps_bc", tag="h0")
                nc.tensor.matmul(ps_bc[:, 0:A], onesr[:],
                                 swrow[:], start=True, stop=True)
                nc.vector.tensor_copy(swb[:], ps_bc[:, 0:A])
                ps_bc2 = hps.tile([128, A], F32, name="ps_bc2", tag="h0")
                nc.tensor.matmul(ps_bc2[:, 0:1], onesf[:], bsc_row[:],
                                 start=True, stop=True)
                nc.vector.tensor_copy(sbb[:], ps_bc2[:, 0:1])

                # h0T (full, all mtiles) and the local c0 slice
                for mt in range(KH):
                    smw_sb = hp.tile([128, KF, B], F32, name=f"smw_{mt}",
                                     tag="smw", bufs=1)
                    nc.sync.dma_start(smw_sb[:], d["smw"][mt])
                    h0_ps = hps.tile([128, A], F32, name=f"h0_{mt}",
                                     tag="h0")
                    for k in range(KF):
                        nc.tensor.matmul(h0_ps[:, 0:128], smw_sb[:, k, :],
                                         meanT[:, k, :],
                                         start=(k == 0), stop=False)
                    nc.tensor.matmul(h0_ps[:, 0:128],
                                     bsm[:, mt * 128:(mt + 1) * 128],
                                     onesf[:], start=False, stop=True)
                    nc.vector.tensor_copy(h2t0[:, mt, :], h0_ps[:, 0:128])

                c0_ps = hps.tile([128, A], F32, name="c0_ps", tag="h0")
                for k in range(KF):
                    lmwc = hp.tile([128, HS], F32, name=f"lmw_{k}",
                                   tag="lmw", bufs=2)
                    nc.sync.dma_start(lmwc[:], d["lmw"][k])
                    nc.tensor.matmul(c0_ps[:, 0:HS], meanT[:, k, :], lmwc[:],
                                     start=(k == 0), stop=False)
                nc.tensor.matmul(c0_ps[:, 0:HS], onesf[:], blm[:],
                                 start=False, stop=True)
                nc.vector.tensor_copy(c_loc[:], c0_ps[:, 0:HS])

        # ================= decode steps =================
        with (
            tc.tile_pool(name="work", bufs=1) as wk,
            tc.tile_pool(name="ps", bufs=1, space="PSUM") as ps,
            tc.tile_pool(name="pslog", bufs=2, space="PSUM") as pslog,
        ):
            h2t = h2t0
            for t in range(T):
                # ---- attention scores (l-shard) ----
                xd_ps = ps.tile([128, A], F32, name=f"xd_{t}", tag="xd")
                for k in range(KH):
                    nc.tensor.matmul(xd_ps[:], h2t[:, k, :], FC1[:, k, :],
                                     start=(k == 0), stop=False)
                nc.tensor.matmul(xd_ps[:], onesr[:], bfc1[:],
                                 start=False, stop=True)
                sc_j = wk.tile([128, LS], F32, name=f"scj_{t}", tag="scj")
                for li in range(LS):
                    comb = wk.tile([128, A], F32, name=f"comb_{t}_{li}",
                                   tag="comb", bufs=2)
                    nc.vector.tensor_tensor(comb[:], XENC[:, li, :], xd_ps[:],
                                            op=ALU.add)
                    nc.scalar.activation(comb[:], comb[:], AF.Tanh)
                    wtd = wk.tile([128, A], BF, name=f"wtd_{t}_{li}",
                                  tag="wtd", bufs=2)
                    nc.gpsimd.tensor_tensor(wtd[:], comb[:], swb[:],
                                            op=ALU.mult)
                    nc.vector.tensor_reduce(sc_j[:, li:li + 1], wtd[:],
                                            mybir.AxisListType.X, ALU.add)
                # + score_b
                nc.vector.tensor_scalar_add(sc_j[:], sc_j[:], sbb[:])
                # AllGather scores
                sc_in = db.tile([128, LS], F32, name=f"sci_{t}", tag="sci")
                sc_out = db.tile([NC * 128, LS], F32, name=f"sco_{t}",
                                 tag="sco", addr_space="Shared")
                nc.sync.dma_start(sc_in[:], sc_j[:])
                nc.gpsimd.collective_compute(
                    "AllGather", ALU.bypass, replica_groups=rg,
                    ins=[sc_in[:]], outs=[sc_out[:]])
                scores = wk.tile([128, L], F32, name=f"scores_{t}",
                                 tag="scores")
                for r in range(NC):
                    nc.sync.dma_start(scores[:, r * LS:(r + 1) * LS],
                                      sc_out[r * 128:(r + 1) * 128, :])

                # ---- softmax over l ----
                nmax = wk.tile([128, 1], F32, name=f"nmax_{t}", tag="nmax")
                nc.vector.tensor_reduce(nmax[:], scores[:],
                                        mybir.AxisListType.X,
                                        ALU.max, negate=True)
                expv = wk.tile([128, L], F32, name=f"expv_{t}", tag="expv")
                nc.scalar.activation(expv[:], scores[:], AF.Exp, bias=nmax[:])
                ssum = wk.tile([128, 1], F32, name=f"ssum_{t}", tag="ssum")
                nc.vector.tensor_reduce(ssum[:], expv[:],
                                        mybir.AxisListType.X, ALU.add)
                rcp = wk.tile([128, 1], F32, name=f"rcp_{t}", tag="rcp")
                nc.vector.reciprocal(rcp[:], ssum[:])
                alpha = wk.tile([128, L], F32, name=f"alpha_{t}", tag="alpha")
                nc.vector.tensor_scalar(alpha[:], expv[:], rcp[:], None,
                                        op0=ALU.mult)
                nc.sync.dma_start(d["alphas_out"][t], alpha[:])
                alpha_b = wk.tile([128, L], BF, name=f"alphab_{t}",
                                  tag="alphab")
                nc.vector.tensor_copy(alpha_b[:], alpha[:])

                # ---- context (f-shard) via diag trick ----
                ctx_ps = ps.tile([128, FS], F32, name=f"ctx_{t}", tag="ctx")
                LH = L // 2
                for half in range(2):
                    diag = wk.tile([128, LH, 128], BF,
                                   name=f"diag_{t}_{half}", tag="diag",
                                   bufs=1)
                    d0, d1 = bass.broadcast_tensor_aps(
                        ideb[:, None, :],
                        alpha_b[:, half * LH:(half + 1) * LH, None])
                    nc.vector.tensor_tensor(diag[:], d0, d1, op=ALU.mult)
                    for li in range(LH):
                        la = half * LH + li
                        nc.tensor.matmul(ctx_ps[:], diag[:, li, :],
                                         XCTX[:, la, :],
                                         start=(la == 0), stop=(la == L - 1))
                ctx_sb = wk.tile([128, FS], F32, name=f"ctxsb_{t}",
                                 tag="ctxsb")
                nc.vector.tensor_copy(ctx_sb[:], ctx_ps[:])
                ctxT_b = wk.tile([128, 2, 128], BF, name=f"ctxT_{t}",
                                 tag="ctxT")
                for k in range(2):
                    ctp = ps.tile([128, 128], F32, name=f"ctp_{t}_{k}",
                                  tag="ctp", bufs=2)
                    nc.tensor.transpose(ctp[:],
                                        ctx_sb[:, k * 128:(k + 1) * 128],
                                        idef[:])
                    nc.vector.tensor_copy(ctxT_b[:, k, :], ctp[:])
                ctx_in = db.tile([2 * 128, B], BF, name=f"cti_{t}", tag="cti")
                ctx_out = db.tile([KF * 128, B], BF, name=f"cto_{t}",
                                  tag="cto", addr_space="Shared")
                for k in range(2):
                    nc.sync.dma_start(ctx_in[k * 128:(k + 1) * 128, :],
                                      ctxT_b[:, k, :])
                nc.gpsimd.collective_compute(
                    "AllGather", ALU.bypass, replica_groups=rg,
                    ins=[ctx_in[:]], outs=[ctx_out[:]])
                CTXT = wk.tile([128, KF, B], BF, name=f"CTXT_{t}", tag="CTXT",
                               bufs=2)
                for k in range(KF):
                    nc.sync.dma_start(CTXT[:, k, :],
                                      ctx_out[k * 128:(k + 1) * 128, :])

                # ---- gates ----
                xs_sb = wk.tile([128, KE, B], BF, name=f"xs_{t}", tag="xs",
                                bufs=2)
                for k in range(KE):
                    nc.sync.dma_start(xs_sb[:, k, :], d["xsT"][t, k])
                g_ps = ps.tile([128, G], F32, name=f"g_{t}", tag="g")
                for k in range(KE):
                    nc.tensor.matmul(g_ps[:], xs_sb[:, k, :], WIH[:, k, :],
                                     start=(k == 0), stop=False)
                for k in range(KF):
                    nc.tensor.matmul(g_ps[:], CTXT[:, k, :], WIH[:, KE + k, :],
                                     start=False, stop=False)
                for k in range(KH):
                    nc.tensor.matmul(g_ps[:], h2t[:, k, :], WHH[:, k, :],
                                     start=False, stop=False)
                nc.tensor.matmul(g_ps[:], onesr[:], bih[:],
                                 start=False, stop=False)
                nc.tensor.matmul(g_ps[:], onesr[:], bhh[:],
                                 start=False, stop=True)

                # ---- pointwise LSTM on the local slice ----
                ifo = wk.tile([128, 3 * HS], F32, name=f"ifo_{t}", tag="ifo")
                nc.scalar.activation(ifo[:, 0:HS], g_ps[:, 0:HS], AF.Sigmoid)
                nc.scalar.activation(ifo[:, HS:2 * HS], g_ps[:, HS:2 * HS],
                                     AF.Sigmoid)
                nc.scalar.activation(ifo[:, 2 * HS:3 * HS],
                                     g_ps[:, 3 * HS:4 * HS], AF.Sigmoid)
                gt = wk.tile([128, HS], F32, name=f"gt_{t}", tag="gt")
                nc.scalar.activation(gt[:], g_ps[:, 2 * HS:3 * HS], AF.Tanh)
                fc = wk.tile([128, HS], F32, name=f"fcm_{t}", tag="fcm")
                nc.vector.tensor_tensor(fc[:], ifo[:, HS:2 * HS], c_loc[:],
                                        op=ALU.mult)
                ig = wk.tile([128, HS], F32, name=f"ig_{t}", tag="ig")
                nc.vector.tensor_tensor(ig[:], ifo[:, 0:HS], gt[:],
                                        op=ALU.mult)
                nc.vector.tensor_tensor(c_loc[:], fc[:], ig[:], op=ALU.add)
                tc_t = wk.tile([128, HS], F32, name=f"tc_{t}", tag="tc")
                nc.scalar.activation(tc_t[:], c_loc[:], AF.Tanh)
                h2_sb = wk.tile([128, HS], F32, name=f"h2_{t}", tag="h2")
                nc.vector.tensor_tensor(h2_sb[:], ifo[:, 2 * HS:3 * HS],
                                        tc_t[:], op=ALU.mult)
                # transpose local h2 slice -> [hs, b], AllGather full h2T
                htp = ps.tile([128, 128], F32, name=f"htp_{t}", tag="ctp",
                              bufs=2)
                nc.tensor.transpose(htp[:], h2_sb[:], idef[:])
                h2T_loc = wk.tile([128, 128], BF, name=f"h2Tl_{t}",
                                  tag="h2Tl")
                nc.vector.tensor_copy(h2T_loc[:], htp[:])
                h_in = db.tile([128, B], BF, name=f"hi_{t}", tag="hi")
                h_out = db.tile([NC * 128, B], BF, name=f"ho_{t}", tag="ho",
                                addr_space="Shared")
                nc.sync.dma_start(h_in[:], h2T_loc[:])
                nc.gpsimd.collective_compute(
                    "AllGather", ALU.bypass, replica_groups=rg,
                    ins=[h_in[:]], outs=[h_out[:]])
                h2t_new = new_h2t(t)
                for k in range(KH):
                    nc.sync.dma_start(h2t_new[:, k, :],
                                      h_out[k * 128:(k + 1) * 128, :])
                h2t = h2t_new

                # ---- logits (vocab shard) ----
                for cch in range(NLOG):
                    lg_ps = pslog.tile([128, VC], F32, name=f"lg_{t}_{cch}",
                                       tag="lg")
                    for k in range(KH):
                        nc.tensor.matmul(
                            lg_ps[:], h2t[:, k, :],
                            OW[:, k, cch * VC:(cch + 1) * VC],
                            start=(k == 0), stop=False)
                    nc.tensor.matmul(lg_ps[:], onesb[:],
                                     bout[:, cch * VC:(cch + 1) * VC],
                                     start=False, stop=True)
                    lst = wk.tile([128, VC], F32, name=f"lst_{t}_{cch}",
                                  tag="lst", bufs=2)
                    if cch % 2 == 0:
                        nc.vector.tensor_copy(lst[:], lg_ps[:])
                    else:
                        nc.scalar.copy(lst[:], lg_ps[:])
                    nc.sync.dma_start(
                        d["logits_out"][t, :, cch * VC:(cch + 1) * VC],
                        lst[:])


def _shard_inputs(X, y, emb, fc1_w, fc1_b, fc2_w, fc2_b, score_w, score_b,
                  sm_w, sm_b, lm_w, lm_b, w_ih, b_ih, w_hh, b_hh, out_w,
                  out_b):
    X = np.asarray(X, np.float32)
    y = np.asarray(y).astype(np.int64)
    emb = np.asarray(emb, np.float32)

    # gathered embeddings, transposed per step: [T, KE, 128, B]
    emb_caps = emb[y[:, :T]]                      # [B, T, E]
    xsT = np.ascontiguousarray(emb_caps.transpose(1, 2, 0))  # [T, E, B]
    xsT = xsT.reshape(T, KE, 128, B).astype(BF16)

    XT = np.ascontiguousarray(X.transpose(2, 0, 1))  # [F, B, L] -> index [f, b, l]

    shared = {
        "xsT": xsT,
        "fc1": np.ascontiguousarray(
            np.asarray(fc1_w, np.float32).reshape(KH, 128, A)).astype(BF16),
        "fc2": np.ascontiguousarray(
            np.asarray(fc2_w, np.float32).reshape(KF, 128, A)),
        "swr": np.asarray(score_w, np.float32).reshape(1, A).copy(),
        "ide_f": np.eye(128, dtype=np.float32),
        "ide_b": np.eye(128, dtype=np.float32).astype(BF16),
        "ones_f": np.ones((1, 128), np.float32),
        "ones_b": np.ones((1, 128), BF16),
        "ones_r": np.ones((1, 128), np.float32),
        "b_fc1": np.asarray(fc1_b, np.float32).reshape(1, A).copy(),
        "b_fc2": np.asarray(fc2_b, np.float32).reshape(1, A).copy(),
        "b_sm": np.asarray(sm_b, np.float32).reshape(1, H).copy(),
        "b_sc": np.asarray(score_b, np.float32).reshape(1, 1).copy(),
        "b_out_full": np.asarray(out_b, np.float32),
        "b_ih_full": np.asarray(b_ih, np.float32),
        "b_hh_full": np.asarray(b_hh, np.float32),
        "b_lm_full": np.asarray(lm_b, np.float32),
    }
    # sm_w: [mt, p, kt, b] where sm_w[kt*128+p, mt*128+b]
    smw = np.asarray(sm_w, np.float32).reshape(KF, 128, KH, 128)
    smw = np.ascontiguousarray(smw.transpose(2, 1, 0, 3))  # [mt, p, kt, b]

    w_ih = np.asarray(w_ih, np.float32)
    w_hh = np.asarray(w_hh, np.float32)
    out_w = np.asarray(out_w, np.float32)
    lm_w = np.asarray(lm_w, np.float32)

    in_maps = []
    for j in range(NC):
        lj = slice(j * LS, (j + 1) * LS)
        fj = slice(j * FS, (j + 1) * FS)
        hj = slice(j * HS, (j + 1) * HS)
        vj = slice(j * VS, (j + 1) * VS)
        gcols = np.concatenate(
            [np.arange(g * H + j * HS, g * H + (j + 1) * HS)
             for g in range(4)])

        # X[:, lj, :]^T as [l, p, kt, b]: element [li, p, kt, b] =
        # X[b, lj0+li, kt*128+p]
        xt = X[:, lj, :].transpose(1, 2, 0)        # [LS, F, B]
        xt = np.ascontiguousarray(xt).reshape(LS, KF, 128, B)
        xt = np.ascontiguousarray(xt.transpose(0, 2, 1, 3))  # [l, p, kt, b]

        m = dict(shared)
        del m["b_out_full"], m["b_ih_full"], m["b_hh_full"], m["b_lm_full"]
        m.update({
            "xt_in": xt,
            "xctx": np.ascontiguousarray(X[:, :, fj]).astype(BF16),
            "wih": np.ascontiguousarray(
                w_ih[:, gcols].reshape(KEF, 128, G)).astype(BF16),
            "whh": np.ascontiguousarray(
                w_hh[:, gcols].reshape(KH, 128, G)).astype(BF16),
            "oww": np.ascontiguousarray(
                out_w[:, vj].reshape(KH, 128, VS)).astype(BF16),
            "smw": smw,
            "lmw": np.ascontiguousarray(lm_w[:, hj].reshape(KF, 128, HS)),
            "b_ih": shared["b_ih_full"][gcols].reshape(1, G).copy(),
            "b_hh": shared["b_hh_full"][gcols].reshape(1, G).copy(),
            "b_out": shared["b_out_full"][vj].reshape(1, VS).astype(BF16),
            "b_lm": shared["b_lm_full"][hj].reshape(1, HS).copy(),
        })
        in_maps.append(m)
    return in_maps


def kernel(**inputs):
    global LAST_RESULT
    in_maps = _shard_inputs(**inputs)
    nc = bacc.Bacc("TRN2", target_bir_lowering=False, debug=False,
                   num_devices=NC)
    _build(nc)
    nc.compile()
    res = run_bass_kernel_spmd(
        nc, in_maps, core_ids=list(range(NC)),
        trace=bool(os.environ.get("KERNEL_TRACE")))
    LAST_RESULT = res
    outs = res.results
    logits = np.concatenate([o["logits_out"] for o in outs], axis=2)
    outputs = np.ascontiguousarray(
        logits.transpose(1, 0, 2)).astype(np.float32)
    weights = np.ascontiguousarray(
        outs[0]["alphas_out"].transpose(1, 0, 2)).astype(np.float32)
    return outputs, weights


def bench_exec(inputs, iters=4):
    """Time repeated device executions of the compiled program (inputs
    device-resident, zero output buffers re-staged outside the timed
    region). Returns min wall-time in ns, an upper bound on HW exec time
    including dispatch overhead."""
    import time
    import jax
    import concourse.mybir as _mybir
    from jax.sharding import Mesh, PartitionSpec
    from jax.experimental.shard_map import shard_map
    from concourse import bass2jax

    in_maps = _shard_inputs(**inputs)
    nc = bacc.Bacc("TRN2", target_bir_lowering=False, debug=False,
                   num_devices=NC)
    _build(nc)
    nc.compile()
    bass2jax.install_neuronx_cc_hook()

    partition_name = (nc.partition_id_tensor.name
                      if nc.partition_id_tensor else None)
    in_names, out_names, out_avals, zero_outs = [], [], [], []
    for alloc in nc.m.functions[0].allocations:
        if not isinstance(alloc, _mybir.MemoryLocationSet):
            continue
        name = alloc.memorylocations[0].name
        if alloc.kind == "ExternalInput":
            if name != partition_name:
                in_names.append(name)
        elif alloc.kind == "ExternalOutput":
            shape = tuple(alloc.tensor_shape)
            dtype = _mybir.dt.np(alloc.dtype)
            out_names.append(name)
            out_avals.append(jax.core.ShapedArray(shape, dtype))
            zero_outs.append(np.zeros(shape, dtype))
    n_params = len(in_names)
    n_outs = len(out_avals)
    in_names = in_names + out_names
    if partition_name is not None:
        in_names.append(partition_name)
    donate = tuple(range(n_params, n_params + n_outs))

    def _body(*args):
        operands = list(args)
        if partition_name is not None:
            operands.append(bass2jax.partition_id_tensor())
        return tuple(bass2jax._bass_exec_p.bind(
            *operands, out_avals=tuple(out_avals), in_names=tuple(in_names),
            out_names=tuple(out_names), lowering_input_output_aliases=(),
            sim_require_finite=True, sim_require_nnan=True, nc=nc))

    devices = jax.devices()[:NC]
    mesh = Mesh(np.asarray(devices), ("core",))
    sharded = jax.jit(
        shard_map(_body, mesh=mesh,
                  in_specs=(PartitionSpec("core"),) * (n_params + n_outs),
                  out_specs=(PartitionSpec("core"),) * n_outs,
                  check_rep=False),
        donate_argnums=donate, keep_unused=True)

    concat_in = [
        np.concatenate([np.asarray(in_maps[c][in_names[i]])
                        for c in range(NC)], axis=0)
        for i in range(n_params)
    ]
    dev_in = [jax.device_put(a) for a in concat_in]
    for a in dev_in:
        a.block_until_ready()

    def fresh_zeros():
        zs = [jax.device_put(np.zeros((NC * z.shape[0], *z.shape[1:]),
                                      z.dtype)) for z in zero_outs]
        for a in zs:
            a.block_until_ready()
        return zs

    # warmup (compiles)
    outs = sharded(*dev_in, *fresh_zeros())
    for o in outs:
        o.block_until_ready()

    best = None
    for _ in range(iters):
        zs = fresh_zeros()
        t0 = time.perf_counter()
        outs = sharded(*dev_in, *zs)
        for o in outs:
            o.block_until_ready()
        dt = time.perf_counter() - t0
        best = dt if best is None else min(best, dt)
        print(f"  bench iter: {dt*1e3:.2f} ms")
    return best * 1e9
